# revision 4
# baseline (speedup 1.0000x reference)
"""Trainium2 Bass kernel for the 4-layer ARMAConv GNN (nn_Net_52587579572464).

Math (per graph, per layer, K=3 stacks):
    h_out = relu(mean_k relu(a @ (x @ W_k) + x @ V_k + b_k))
Restructured:
    xa = a @ x                      (shared across the K stacks: a(xW) == (ax)W)
    Z  = [x ; xa] @ [[V_k];[W_k]]   (one matmul, 3 stacks stacked into columns)
    h_out = sum_k relu(Z_k + b_k)   (outer relu is a no-op on a sum of relus;
                                     the 1/3 mean is folded into the next
                                     layer's weights / the dense head)

Device mapping per NeuronCore (16 graphs each, 8 cores data-parallel):
    - activations kept feature-major (hT: [C,400]) for the channel matmul
      (stationary = U 128x128 blocks, moving = hT rows, float32r @ 1 cyc/row)
    - node-major copy (h: [512pad,C]) produced by PE transposes, used as the
      stationary operand of the GSO matmul (moving = aT, nodes padded 400->512
      with zeros so garbage partitions never contribute)
    - U2..U4 (6MB each) streamed from HBM per graph-pair; U1/aT/Wd resident.
"""

import sys

for _p in ("/opt/trn_rl_repo", "/root/.axon_site/_ro/trn_rl_repo"):
    if _p not in sys.path:
        sys.path.insert(0, _p)

import numpy as np

import concourse.bass as bass
import concourse.bacc as bacc
import concourse.tile as tile
from concourse import mybir
from concourse.bass_utils import run_bass_kernel_spmd
from concourse.masks import make_identity

F32 = mybir.dt.float32
F32R = mybir.dt.float32r
RELU = mybir.ActivationFunctionType.Relu

NCORES = 8
B = 128
G = B // NCORES          # graphs per core
N = 400                  # nodes
NP = 512                 # nodes padded (contraction dim of the GSO matmul)
F = 240                  # input features
FP = 256                 # input features padded
C = 512                  # hidden channels
K = 3                    # ARMA stacks
L = 480                  # labels
NMT = NP // 128          # 4 node m-tiles
NCC = C // 128           # 4 channel chunks
NJ = K * NCC             # 12 cout blocks of 128


def _build_nc():
    nc = bacc.Bacc("TRN2", target_bir_lowering=False)

    xt_d = nc.dram_tensor("xt", [G, 128, 2, N], F32, kind="ExternalInput")
    xn_d = nc.dram_tensor("xn", [G, 128, NMT, 2, 128], F32, kind="ExternalInput")
    at_d = nc.dram_tensor("at", [128, NMT, N], F32, kind="ExternalInput")
    u1_d = nc.dram_tensor("u1", [NJ, 128, 4, 128], F32, kind="ExternalInput")
    u_ds = [
        nc.dram_tensor(f"u{l}", [NJ, 128, 8, 128], F32, kind="ExternalInput")
        for l in (2, 3, 4)
    ]
    wd_d = nc.dram_tensor("wd", [128, NCC, L], F32, kind="ExternalInput")
    bias_d = nc.dram_tensor("bias", [128, 4, NJ], F32, kind="ExternalInput")
    bdb_d = nc.dram_tensor("bdb", [128, L], F32, kind="ExternalInput")
    y_d = nc.dram_tensor("y", [G, N, L], F32, kind="ExternalOutput")

    from contextlib import ExitStack

    with tile.TileContext(nc) as tc, ExitStack() as ctx:
        const = ctx.enter_context(tc.tile_pool(name="const", bufs=1))
        xpool = ctx.enter_context(tc.tile_pool(name="xpool", bufs=2))
        upool = ctx.enter_context(tc.tile_pool(name="upool", bufs=3))
        hpool = ctx.enter_context(tc.tile_pool(name="hpool", bufs=2))
        rpool = ctx.enter_context(tc.tile_pool(name="rpool", bufs=2))
        ypool = ctx.enter_context(tc.tile_pool(name="ypool", bufs=3))
        ps_gso = ctx.enter_context(tc.tile_pool(name="ps_gso", bufs=2, space="PSUM"))
        ps_chan = ctx.enter_context(tc.tile_pool(name="ps_chan", bufs=3, space="PSUM"))
        ps_tr = ctx.enter_context(tc.tile_pool(name="ps_tr", bufs=2, space="PSUM"))

        # ---- resident constants ----
        ident = const.tile([128, 128], F32)
        make_identity(nc, ident)
        at_sb = const.tile([128, NMT, N], F32R)
        nc.sync.dma_start(out=at_sb[:], in_=at_d[:].bitcast(F32R))
        u1_sb = const.tile([128, NJ, 4, 128], F32R)
        nc.sync.dma_start(
            out=u1_sb[:], in_=u1_d.rearrange("j p f q -> p j f q").bitcast(F32R)
        )
        wd_sb = const.tile([128, NCC, L], F32R)
        nc.sync.dma_start(out=wd_sb[:], in_=wd_d[:].bitcast(F32R))
        bias_sb = const.tile([128, 4, NJ], F32)
        nc.sync.dma_start(out=bias_sb[:], in_=bias_d[:])
        bdb_sb = const.tile([128, L], F32)
        nc.sync.dma_start(out=bdb_sb[:], in_=bdb_d[:])

        def gso_layer(stat_blocks, nfc, xa_out):
            # xa.T[fchunk] = sum_mt stat[mt,fchunk].T @ aT[mt]   (feature-major out)
            for fc in range(nfc):
                ps = ps_gso.tile([128, N], F32, tag="gso")
                for mt in range(NMT):
                    nc.tensor.matmul(
                        ps[:],
                        stat_blocks(mt, fc),
                        at_sb[:, mt, :],
                        start=(mt == 0),
                        stop=(mt == NMT - 1),
                    )
                nc.scalar.copy(xa_out[:, fc, :], ps[:])

        def chan_layer(g, li, u_block, nf, moving, hT_out):
            # Z.T[jj] = sum_f U[f,jj].T @ xcatT[f] ; relu+bias ; sum the K stacks
            for cc in range(NCC):
                rk = []
                for k in range(K):
                    jj = k * NCC + cc
                    ub = u_block(jj)
                    ps = ps_chan.tile([128, 512], F32, tag="chan")
                    for f in range(nf):
                        nc.tensor.matmul(
                            ps[:, :N],
                            ub(f),
                            moving(f),
                            start=(f == 0),
                            stop=(f == nf - 1),
                        )
                    r = rpool.tile([128, N], F32, tag=f"r{g % 2}_{k}")
                    nc.scalar.activation(
                        r[:], ps[:, :N], RELU, bias=bias_sb[:, li, jj : jj + 1]
                    )
                    rk.append(r)
                nc.vector.tensor_add(hT_out[:, cc, :N], rk[0][:], rk[1][:])
                nc.vector.tensor_add(hT_out[:, cc, :N], hT_out[:, cc, :N], rk[2][:])

        def transpose_h(hT, h_node):
            # hT [128, NCC, 512] (cols 400:512 zeroed) -> h_node [128, NMT, NCC, 128]
            for cc in range(NCC):
                for mt in range(NMT):
                    ps = ps_tr.tile([128, 128], F32, tag="tr")
                    nc.tensor.transpose(
                        ps[:], hT[:, cc, mt * 128 : (mt + 1) * 128].bitcast(F32), ident[:]
                    )
                    nc.vector.tensor_copy(h_node[:, mt, cc, :], ps[:])

        for pp in range(G // 2):
            gs = (2 * pp, 2 * pp + 1)

            # ---------- layer 1 ----------
            xt_sb = {}
            xn_sb = {}
            xa1 = {}
            hT = {}
            hn = {}
            for g in gs:
                xt_sb[g] = xpool.tile([128, 2, N], F32R, tag=f"xt{g % 2}", name=f"xt_g{g}")
                nc.sync.dma_start(out=xt_sb[g][:], in_=xt_d[g].bitcast(F32R))
                xn_sb[g] = xpool.tile([128, NMT, 2, 128], F32R, tag=f"xn{g % 2}", name=f"xn_g{g}")
                nc.sync.dma_start(out=xn_sb[g][:], in_=xn_d[g].bitcast(F32R))
                xa1[g] = xpool.tile([128, 2, N], F32R, tag=f"xa1_{g % 2}", name=f"xa1_g{g}")
                gso_layer(
                    lambda mt, fc, g=g: xn_sb[g][:, mt, fc, :], 2, xa1[g]
                )
            for g in gs:
                hT[g] = hpool.tile([128, NCC, 512], F32R, tag=f"hT{g % 2}", name=f"hT1_g{g}")
                chan_layer(
                    g,
                    0,
                    lambda jj: (lambda f, jj=jj: u1_sb[:, jj, f, :]),
                    4,
                    lambda f, g=g: (
                        xt_sb[g][:, f, :] if f < 2 else xa1[g][:, f - 2, :]
                    ),
                    hT[g],
                )
                for cc in range(NCC):
                    nc.vector.memset(hT[g][:, cc, N:].bitcast(F32), 0.0)
                hn[g] = hpool.tile([128, NMT, NCC, 128], F32R, tag=f"hn{g % 2}", name=f"hn1_g{g}")
                transpose_h(hT[g], hn[g])

            # ---------- layers 2..4 ----------
            for li, u_d in zip((1, 2, 3), u_ds):
                xa = {}
                for g in gs:
                    xa[g] = xpool.tile([128, NCC, N], F32R, tag=f"xa_{g % 2}", name=f"xa_g{g}_l{li}")
                    gso_layer(
                        lambda mt, cc, g=g: hn[g][:, mt, cc, :], NCC, xa[g]
                    )
                hT_new = {}
                hn_new = {}
                for g in gs:
                    hT_new[g] = hpool.tile([128, NCC, 512], F32R, tag=f"hT{g % 2}", name=f"hT_g{g}_l{li}")

                def u_block(jj, u_d=u_d):
                    ub = upool.tile([128, 8, 128], F32R, tag="ublk", name=f"ublk_{jj}")
                    nc.sync.dma_start(out=ub[:], in_=u_d[jj].bitcast(F32R))
                    return lambda f: ub[:, f, :]

                for cc in range(NCC):
                    rk = {g: [] for g in gs}
                    for k in range(K):
                        jj = k * NCC + cc
                        ubf = u_block(jj)
                        for g in gs:
                            ps = ps_chan.tile([128, 512], F32, tag="chan")
                            for f in range(8):
                                mv = (
                                    hT[g][:, f, :N]
                                    if f < NCC
                                    else xa[g][:, f - NCC, :]
                                )
                                nc.tensor.matmul(
                                    ps[:, :N],
                                    ubf(f),
                                    mv,
                                    start=(f == 0),
                                    stop=(f == 7),
                                )
                            r = rpool.tile([128, N], F32, tag=f"r{g % 2}_{k}")
                            nc.scalar.activation(
                                r[:], ps[:, :N], RELU, bias=bias_sb[:, li, jj : jj + 1]
                            )
                            rk[g].append(r)
                    for g in gs:
                        nc.vector.tensor_add(
                            hT_new[g][:, cc, :N], rk[g][0][:], rk[g][1][:]
                        )
                        nc.vector.tensor_add(
                            hT_new[g][:, cc, :N], hT_new[g][:, cc, :N], rk[g][2][:]
                        )
                if li < 3:
                    for g in gs:
                        for cc in range(NCC):
                            nc.vector.memset(hT_new[g][:, cc, N:].bitcast(F32), 0.0)
                        hn_new[g] = hpool.tile([128, NMT, NCC, 128], F32R, tag=f"hn{g % 2}", name=f"hn_g{g}_l{li}")
                        transpose_h(hT_new[g], hn_new[g])
                    hn = hn_new
                hT = hT_new

            # ---------- dense head ----------
            for g in gs:
                for nt in range(NMT):
                    w = 128 if nt < 3 else N - 3 * 128
                    ps = ps_chan.tile([128, 512], F32, tag="chan")
                    for cc in range(NCC):
                        nc.tensor.matmul(
                            ps[:w, :L],
                            hT[g][:, cc, nt * 128 : nt * 128 + w],
                            wd_sb[:, cc, :],
                            start=(cc == 0),
                            stop=(cc == NCC - 1),
                        )
                    y_sb = ypool.tile([128, L], F32, tag="y")
                    nc.vector.tensor_add(y_sb[:w, :], ps[:w, :L], bdb_sb[:w, :])
                    nc.sync.dma_start(
                        out=y_d[g, nt * 128 : nt * 128 + w, :], in_=y_sb[:w, :]
                    )

    nc.compile()
    return nc


def _pack_inputs(x, a, Ws, Vs, bs, Wd, bd):
    """Host-side packing into the per-core DMA-friendly layouts."""
    x = np.asarray(x, np.float32)
    a = np.asarray(a, np.float32)

    # aT padded: [p, mt, n] = a[n, mt*128+p], zero for node >= 400
    at_pack = np.zeros((NP, N), np.float32)
    at_pack[:N, :] = a.T
    at_pack = np.ascontiguousarray(at_pack.reshape(NMT, 128, N).transpose(1, 0, 2))

    # U1: rows [V1 pad 256 ; W1 pad 256], cols k-major (k*512 + c)
    U1 = np.zeros((512, K * C), np.float32)
    for k in range(K):
        U1[:F, k * C : (k + 1) * C] = Vs[0][k]
        U1[FP : FP + F, k * C : (k + 1) * C] = Ws[0][k]
    u1_pack = np.ascontiguousarray(
        U1.reshape(4, 128, NJ, 128).transpose(2, 1, 0, 3)
    )

    u_packs = []
    for l in range(1, 4):
        U = np.empty((2 * C, K * C), np.float32)
        for k in range(K):
            U[:C, k * C : (k + 1) * C] = Vs[l][k] / 3.0
            U[C:, k * C : (k + 1) * C] = Ws[l][k] / 3.0
        u_packs.append(
            np.ascontiguousarray(U.reshape(8, 128, NJ, 128).transpose(2, 1, 0, 3))
        )

    wd_pack = np.ascontiguousarray(
        (np.asarray(Wd, np.float32) / 3.0).reshape(NCC, 128, L).transpose(1, 0, 2)
    )

    bias_pack = np.zeros((128, 4, NJ), np.float32)
    for li in range(4):
        for jj in range(NJ):
            k, cc = divmod(jj, NCC)
            bias_pack[:, li, jj] = bs[li][k, cc * 128 : (cc + 1) * 128]

    bdb_pack = np.ascontiguousarray(
        np.broadcast_to(np.asarray(bd, np.float32), (128, L))
    )

    in_maps = []
    for c in range(NCORES):
        xs = x[c * G : (c + 1) * G]  # (G, 400, 240)
        x_pad = np.zeros((G, NP, FP), np.float32)
        x_pad[:, :N, :F] = xs
        xn_pack = np.ascontiguousarray(
            x_pad.reshape(G, NMT, 128, 2, 128).transpose(0, 2, 1, 3, 4)
        )
        xt_pack = np.ascontiguousarray(
            x_pad[:, :N, :].transpose(0, 2, 1).reshape(G, 2, 128, N).transpose(0, 2, 1, 3)
        )
        in_maps.append(
            {
                "xt": xt_pack,
                "xn": xn_pack,
                "at": at_pack,
                "u1": u1_pack,
                "u2": u_packs[0],
                "u3": u_packs[1],
                "u4": u_packs[2],
                "wd": wd_pack,
                "bias": bias_pack,
                "bdb": bdb_pack,
            }
        )
    return in_maps


_NC_CACHE = {}


def _get_nc():
    if "nc" not in _NC_CACHE:
        _NC_CACHE["nc"] = _build_nc()
    return _NC_CACHE["nc"]


def kernel(
    x, a, W1, V1, b1, W2, V2, b2, W3, V3, b3, W4, V4, b4, Wd, bd
) -> np.ndarray:
    in_maps = _pack_inputs(
        x,
        a,
        [np.asarray(W, np.float32) for W in (W1, W2, W3, W4)],
        [np.asarray(V, np.float32) for V in (V1, V2, V3, V4)],
        [np.asarray(b, np.float32) for b in (b1, b2, b3, b4)],
        Wd,
        bd,
    )
    nc = _get_nc()
    res = run_bass_kernel_spmd(nc, in_maps, core_ids=list(range(NCORES)))
    return np.concatenate([res.results[c]["y"] for c in range(NCORES)], axis=0)


# revision 7
# speedup vs baseline: 21.2700x; 21.2700x over previous
"""Trainium2 Bass kernel for the 4-layer ARMAConv GNN (nn_Net_52587579572464).

Math (per graph, per layer, K=3 stacks):
    h_out = relu(mean_k relu(a @ (x @ W_k) + x @ V_k + b_k))
Restructured:
    xa = a @ x                      (shared across the K stacks: a(xW) == (ax)W)
    Z  = [x ; xa] @ [[V_k];[W_k]]   (one matmul, 3 stacks stacked into columns)
    h_out = sum_k relu(Z_k + b_k)   (outer relu is a no-op on a sum of relus;
                                     the 1/3 mean is folded into the next
                                     layer's weights / the dense head)

Device mapping per NeuronCore (16 graphs each, 8 cores data-parallel):
    - activations kept feature-major (hT: [C,400]) for the channel matmul
      (stationary = U 128x128 blocks, moving = hT rows, float32r)
    - node-major copy (h: [512pad,C]) produced by PE transposes, used as the
      stationary operand of the GSO matmul (moving = aT, nodes padded 400->512
      with zeros so garbage partitions never contribute)
    - U1..U4 column-blocks streamed from HBM per graph-pair; aT/Wd resident.
"""

import sys

for _p in ("/opt/trn_rl_repo", "/root/.axon_site/_ro/trn_rl_repo"):
    if _p not in sys.path:
        sys.path.insert(0, _p)

from contextlib import ExitStack, nullcontext

import numpy as np

import concourse.bass as bass
import concourse.bacc as bacc
import concourse.tile as tile
from concourse import mybir
from concourse.bass_utils import run_bass_kernel_spmd
from concourse.masks import make_identity

F32 = mybir.dt.float32
F32R = mybir.dt.float32r
RELU = mybir.ActivationFunctionType.Relu

NCORES = 8
B = 128
G = B // NCORES          # graphs per core
N = 400                  # nodes
NP = 512                 # nodes padded (contraction dim of the GSO matmul)
F = 240                  # input features
FP = 256                 # input features padded
C = 512                  # hidden channels
K = 3                    # ARMA stacks
L = 480                  # labels
NMT = NP // 128          # 4 node m-tiles
NCC = C // 128           # 4 channel chunks
NJ = K * NCC             # 12 cout blocks of 128


def _build_nc(reps=1):
    nc = bacc.Bacc("TRN2", target_bir_lowering=False)

    xt_d = nc.dram_tensor("xt", [G, 128, 2, N], F32, kind="ExternalInput")
    xn_d = nc.dram_tensor("xn", [G, 128, NMT, 2, 128], F32, kind="ExternalInput")
    at_d = nc.dram_tensor("at", [128, NMT, N], F32, kind="ExternalInput")
    u_ds = [
        nc.dram_tensor(f"u{l}", [NJ, 128, 4 if l == 1 else 8, 128], F32,
                       kind="ExternalInput")
        for l in (1, 2, 3, 4)
    ]
    wd_d = nc.dram_tensor("wd", [128, NCC, L], F32, kind="ExternalInput")
    bias_d = nc.dram_tensor("bias", [128, 4, NJ], F32, kind="ExternalInput")
    bdb_d = nc.dram_tensor("bdb", [128, L], F32, kind="ExternalInput")
    y_d = nc.dram_tensor("y", [G, N, L], F32, kind="ExternalOutput")

    with tile.TileContext(nc) as tc, ExitStack() as ctx:
        const = ctx.enter_context(tc.tile_pool(name="const", bufs=1))
        xpool = ctx.enter_context(tc.tile_pool(name="xpool", bufs=2))
        upool = ctx.enter_context(tc.tile_pool(name="upool", bufs=5))
        hpool = ctx.enter_context(tc.tile_pool(name="hpool", bufs=2))
        rpool = ctx.enter_context(tc.tile_pool(name="rpool", bufs=2))
        ypool = ctx.enter_context(tc.tile_pool(name="ypool", bufs=3))
        ps_gso = ctx.enter_context(tc.tile_pool(name="ps_gso", bufs=2, space="PSUM"))
        ps_chan = ctx.enter_context(tc.tile_pool(name="ps_chan", bufs=3, space="PSUM"))
        ps_tr = ctx.enter_context(tc.tile_pool(name="ps_tr", bufs=2, space="PSUM"))

        # ---- resident constants ----
        ident = const.tile([128, 128], F32)
        make_identity(nc, ident)
        at_sb = const.tile([128, NMT, N], F32R)
        nc.sync.dma_start(out=at_sb[:], in_=at_d[:].bitcast(F32R))
        wd_sb = const.tile([128, NCC, L], F32R)
        nc.sync.dma_start(out=wd_sb[:], in_=wd_d[:].bitcast(F32R))
        bias_sb = const.tile([128, 4, NJ], F32)
        nc.sync.dma_start(out=bias_sb[:], in_=bias_d[:])
        bdb_sb = const.tile([128, L], F32)
        nc.sync.dma_start(out=bdb_sb[:], in_=bdb_d[:])

        def gso_layer(g, stat_blocks, nfc, xa_out):
            # xa.T[fchunk] = sum_mt stat[mt,fchunk].T @ aT[mt]   (feature-major out)
            for fc in range(nfc):
                ps = ps_gso.tile([128, N], F32, tag="gso", name=f"psg_{g}_{fc}")
                for mt in range(NMT):
                    nc.tensor.matmul(
                        ps[:],
                        stat_blocks(mt, fc),
                        at_sb[:, mt, :],
                        start=(mt == 0),
                        stop=(mt == NMT - 1),
                    )
                nc.scalar.copy(xa_out[:, fc, :], ps[:])

        def chan_layer(gs, li, u_d, nf, moving, hT_out):
            # Z.T[jj] = sum_f U[f,jj].T @ xcatT[f] ; relu+bias ; sum K stacks.
            # f outer / graph inner so the stationary U block is reused for
            # both graphs of the pair back-to-back.
            for cc in range(NCC):
                rk = {g: [] for g in gs}
                for k in range(K):
                    jj = k * NCC + cc
                    ub = upool.tile([128, nf, 128], F32R, tag="ublk",
                                    name=f"ublk_l{li}_{jj}")
                    nc.sync.dma_start(out=ub[:], in_=u_d[jj].bitcast(F32R))
                    pss = {
                        g: ps_chan.tile([128, 512], F32, tag="chan",
                                        name=f"psc_{g}_{jj}")
                        for g in gs
                    }
                    for f in range(nf):
                        for g in gs:
                            nc.tensor.matmul(
                                pss[g][:, :N],
                                ub[:, f, :],
                                moving(g, f),
                                start=(f == 0),
                                stop=(f == nf - 1),
                            )
                    for g in gs:
                        r = rpool.tile([128, N], F32, tag=f"r{g % 2}_{k}",
                                       name=f"r_{g}_{k}")
                        nc.scalar.activation(
                            r[:], pss[g][:, :N], RELU,
                            bias=bias_sb[:, li, jj : jj + 1],
                        )
                        rk[g].append(r)
                for g in gs:
                    nc.vector.tensor_add(hT_out[g][:, cc, :N], rk[g][0][:], rk[g][1][:])
                    nc.vector.tensor_add(
                        hT_out[g][:, cc, :N], hT_out[g][:, cc, :N], rk[g][2][:]
                    )

        def transpose_h(hT, h_node):
            # hT [128, NCC, 512] (cols 400:512 zeroed) -> h_node [128, NMT, NCC, 128]
            for cc in range(NCC):
                for mt in range(NMT):
                    ps = ps_tr.tile([128, 128], F32, tag="tr")
                    nc.tensor.transpose(
                        ps[:], hT[:, cc, mt * 128 : (mt + 1) * 128].bitcast(F32),
                        ident[:],
                    )
                    nc.vector.tensor_copy(h_node[:, mt, cc, :], ps[:])

        rep_ctx = tc.For_i(0, reps, 1) if reps > 1 else nullcontext()
        with rep_ctx:
         for pp in range(G // 2):
            gs = (2 * pp, 2 * pp + 1)

            # ---------- layer 1 ----------
            xt_sb, xn_sb, xa1, hT, hn = {}, {}, {}, {}, {}
            for g in gs:
                xt_sb[g] = xpool.tile([128, 2, N], F32R, tag=f"xt{g % 2}",
                                      name=f"xt_g{g}")
                nc.sync.dma_start(out=xt_sb[g][:], in_=xt_d[g].bitcast(F32R))
                xn_sb[g] = xpool.tile([128, NMT, 2, 128], F32R, tag=f"xn{g % 2}",
                                      name=f"xn_g{g}")
                nc.sync.dma_start(out=xn_sb[g][:], in_=xn_d[g].bitcast(F32R))
                xa1[g] = xpool.tile([128, NCC, N], F32R, tag=f"xa_{g % 2}",
                                    name=f"xa1_g{g}")
                gso_layer(g, lambda mt, fc, g=g: xn_sb[g][:, mt, fc, :], 2, xa1[g])
                hT[g] = hpool.tile([128, NCC, 512], F32R, tag=f"hT{g % 2}",
                                   name=f"hT1_g{g}")
            chan_layer(
                gs, 0, u_ds[0], 4,
                lambda g, f: xt_sb[g][:, f, :] if f < 2 else xa1[g][:, f - 2, :],
                hT,
            )
            for g in gs:
                for cc in range(NCC):
                    nc.vector.memset(hT[g][:, cc, N:].bitcast(F32), 0.0)
                hn[g] = hpool.tile([128, NMT, NCC, 128], F32R, tag=f"hn{g % 2}",
                                   name=f"hn1_g{g}")
                transpose_h(hT[g], hn[g])

            # ---------- layers 2..4 ----------
            for li, u_d in zip((1, 2, 3), u_ds[1:]):
                xa, hT_new, hn_new = {}, {}, {}
                for g in gs:
                    xa[g] = xpool.tile([128, NCC, N], F32R, tag=f"xa_{g % 2}",
                                       name=f"xa_g{g}_l{li}")
                    gso_layer(g, lambda mt, cc, g=g: hn[g][:, mt, cc, :], NCC, xa[g])
                    hT_new[g] = hpool.tile([128, NCC, 512], F32R, tag=f"hT{g % 2}",
                                           name=f"hT_g{g}_l{li}")
                chan_layer(
                    gs, li, u_d, 8,
                    lambda g, f: hT[g][:, f, :N] if f < NCC else xa[g][:, f - NCC, :],
                    hT_new,
                )
                if li < 3:
                    for g in gs:
                        for cc in range(NCC):
                            nc.vector.memset(hT_new[g][:, cc, N:].bitcast(F32), 0.0)
                        hn_new[g] = hpool.tile([128, NMT, NCC, 128], F32R,
                                               tag=f"hn{g % 2}", name=f"hn_g{g}_l{li}")
                        transpose_h(hT_new[g], hn_new[g])
                    hn = hn_new
                hT = hT_new

            # ---------- dense head ----------
            for g in gs:
                for nt in range(NMT):
                    w = 128 if nt < 3 else N - 3 * 128
                    ps = ps_chan.tile([128, 512], F32, tag="chan",
                                      name=f"psy_{g}_{nt}")
                    for cc in range(NCC):
                        nc.tensor.matmul(
                            ps[:w, :L],
                            hT[g][:, cc, nt * 128 : nt * 128 + w],
                            wd_sb[:, cc, :],
                            start=(cc == 0),
                            stop=(cc == NCC - 1),
                        )
                    y_sb = ypool.tile([128, L], F32, tag="y", name=f"y_{g}_{nt}")
                    nc.vector.tensor_add(y_sb[:w, :], ps[:w, :L], bdb_sb[:w, :])
                    nc.sync.dma_start(
                        out=y_d[g, nt * 128 : nt * 128 + w, :], in_=y_sb[:w, :]
                    )

    nc.compile()
    return nc


def _pack_inputs(x, a, Ws, Vs, bs, Wd, bd):
    """Host-side packing into the per-core DMA-friendly layouts."""
    x = np.asarray(x, np.float32)
    a = np.asarray(a, np.float32)

    # aT padded: [p, mt, n] = a[n, mt*128+p], zero for node >= 400
    at_pack = np.zeros((NP, N), np.float32)
    at_pack[:N, :] = a.T
    at_pack = np.ascontiguousarray(at_pack.reshape(NMT, 128, N).transpose(1, 0, 2))

    # U1: rows [V1 pad 256 ; W1 pad 256], cols k-major (k*512 + c)
    U1 = np.zeros((512, K * C), np.float32)
    for k in range(K):
        U1[:F, k * C : (k + 1) * C] = Vs[0][k]
        U1[FP : FP + F, k * C : (k + 1) * C] = Ws[0][k]
    u_packs = [
        np.ascontiguousarray(U1.reshape(4, 128, NJ, 128).transpose(2, 1, 0, 3))
    ]
    for l in range(1, 4):
        U = np.empty((2 * C, K * C), np.float32)
        for k in range(K):
            U[:C, k * C : (k + 1) * C] = Vs[l][k] / 3.0
            U[C:, k * C : (k + 1) * C] = Ws[l][k] / 3.0
        u_packs.append(
            np.ascontiguousarray(U.reshape(8, 128, NJ, 128).transpose(2, 1, 0, 3))
        )

    wd_pack = np.ascontiguousarray(
        (np.asarray(Wd, np.float32) / 3.0).reshape(NCC, 128, L).transpose(1, 0, 2)
    )

    bias_pack = np.zeros((128, 4, NJ), np.float32)
    for li in range(4):
        for jj in range(NJ):
            k, cc = divmod(jj, NCC)
            bias_pack[:, li, jj] = bs[li][k, cc * 128 : (cc + 1) * 128]

    bdb_pack = np.ascontiguousarray(
        np.broadcast_to(np.asarray(bd, np.float32), (128, L))
    )

    in_maps = []
    for c in range(NCORES):
        xs = x[c * G : (c + 1) * G]  # (G, 400, 240)
        x_pad = np.zeros((G, NP, FP), np.float32)
        x_pad[:, :N, :F] = xs
        xn_pack = np.ascontiguousarray(
            x_pad.reshape(G, NMT, 128, 2, 128).transpose(0, 2, 1, 3, 4)
        )
        xt_pack = np.ascontiguousarray(
            x_pad[:, :N, :].transpose(0, 2, 1).reshape(G, 2, 128, N).transpose(0, 2, 1, 3)
        )
        in_maps.append(
            {
                "xt": xt_pack,
                "xn": xn_pack,
                "at": at_pack,
                "u1": u_packs[0],
                "u2": u_packs[1],
                "u3": u_packs[2],
                "u4": u_packs[3],
                "wd": wd_pack,
                "bias": bias_pack,
                "bdb": bdb_pack,
            }
        )
    return in_maps


_NC_CACHE = {}


def _get_nc(reps=1):
    key = f"nc{reps}"
    if key not in _NC_CACHE:
        _NC_CACHE[key] = _build_nc(reps)
    return _NC_CACHE[key]


def kernel(
    x, a, W1, V1, b1, W2, V2, b2, W3, V3, b3, W4, V4, b4, Wd, bd
) -> np.ndarray:
    in_maps = _pack_inputs(
        x,
        a,
        [np.asarray(W, np.float32) for W in (W1, W2, W3, W4)],
        [np.asarray(V, np.float32) for V in (V1, V2, V3, V4)],
        [np.asarray(b, np.float32) for b in (b1, b2, b3, b4)],
        Wd,
        bd,
    )
    nc = _get_nc()
    res = run_bass_kernel_spmd(nc, in_maps, core_ids=list(range(NCORES)))
    return np.concatenate([res.results[c]["y"] for c in range(NCORES)], axis=0)


# revision 8
# speedup vs baseline: 52.9569x; 2.4897x over previous
"""Trainium2 Bass kernel for the 4-layer ARMAConv GNN (nn_Net_52587579572464).

Math (per graph, per layer, K=3 stacks):
    h_out = relu(mean_k relu(a @ (x @ W_k) + x @ V_k + b_k))
Restructured:
    xa = a @ x                      (shared across the K stacks: a(xW) == (ax)W)
    Z  = [x ; xa] @ [[V_k];[W_k]]   (one matmul, 3 stacks stacked into columns)
    h_out = sum_k relu(Z_k + b_k)   (outer relu is a no-op on a sum of relus;
                                     the 1/3 mean is folded into the next
                                     layer's weights / the dense head)

Device mapping per NeuronCore (16 graphs each, 8 cores data-parallel):
    - activations kept feature-major (hT: [C,400]) for the channel matmul
      (stationary = U 128x128 blocks, moving = hT rows, float32r)
    - node-major copy (h: [512pad,C]) produced by PE transposes, used as the
      stationary operand of the GSO matmul (moving = aT, nodes padded 400->512
      with zeros so garbage partitions never contribute)
    - U1..U4 column-blocks streamed from HBM per graph-pair; aT/Wd resident.
"""

import sys

for _p in ("/opt/trn_rl_repo", "/root/.axon_site/_ro/trn_rl_repo"):
    if _p not in sys.path:
        sys.path.insert(0, _p)

from contextlib import ExitStack, nullcontext

import numpy as np

import concourse.bass as bass
import concourse.bacc as bacc
import concourse.tile as tile
from concourse import mybir
from concourse.bass_utils import run_bass_kernel_spmd
from concourse.masks import make_identity

F32 = mybir.dt.float32
F32R = mybir.dt.float32r
RELU = mybir.ActivationFunctionType.Relu

NCORES = 8
B = 128
G = B // NCORES          # graphs per core
N = 400                  # nodes
NP = 512                 # nodes padded (contraction dim of the GSO matmul)
F = 240                  # input features
FP = 256                 # input features padded
C = 512                  # hidden channels
K = 3                    # ARMA stacks
L = 480                  # labels
NMT = NP // 128          # 4 node m-tiles
NCC = C // 128           # 4 channel chunks
NJ = K * NCC             # 12 cout blocks of 128


def _build_nc(reps=1):
    nc = bacc.Bacc("TRN2", target_bir_lowering=False)

    xt_d = nc.dram_tensor("xt", [G, 128, 2, N], F32, kind="ExternalInput")
    xn_d = nc.dram_tensor("xn", [G, 128, NMT, 2, 128], F32, kind="ExternalInput")
    at_d = nc.dram_tensor("at", [128, NMT, N], F32, kind="ExternalInput")
    u_ds = [
        nc.dram_tensor(f"u{l}", [NJ, 128, 4 if l == 1 else 8, 128], F32,
                       kind="ExternalInput")
        for l in (1, 2, 3, 4)
    ]
    wd_d = nc.dram_tensor("wd", [128, NCC, L], F32, kind="ExternalInput")
    bias_d = nc.dram_tensor("bias", [128, 4, NJ], F32, kind="ExternalInput")
    bdb_d = nc.dram_tensor("bdb", [128, L], F32, kind="ExternalInput")
    y_d = nc.dram_tensor("y", [G, N, L], F32, kind="ExternalOutput")

    with tile.TileContext(nc) as tc, ExitStack() as ctx:
        const = ctx.enter_context(tc.tile_pool(name="const", bufs=1))
        xpool = ctx.enter_context(tc.tile_pool(name="xpool", bufs=2))
        upool = ctx.enter_context(tc.tile_pool(name="upool", bufs=5))
        hpool = ctx.enter_context(tc.tile_pool(name="hpool", bufs=2))
        rpool = ctx.enter_context(tc.tile_pool(name="rpool", bufs=2))
        ypool = ctx.enter_context(tc.tile_pool(name="ypool", bufs=3))
        ps_gso = ctx.enter_context(tc.tile_pool(name="ps_gso", bufs=2, space="PSUM"))
        ps_chan = ctx.enter_context(tc.tile_pool(name="ps_chan", bufs=3, space="PSUM"))
        ps_tr = ctx.enter_context(tc.tile_pool(name="ps_tr", bufs=2, space="PSUM"))

        # ---- resident constants ----
        ident = const.tile([128, 128], F32)
        make_identity(nc, ident)
        at_sb = const.tile([128, NMT, N], F32R)
        nc.sync.dma_start(out=at_sb[:], in_=at_d[:].bitcast(F32R))
        wd_sb = const.tile([128, NCC, L], F32R)
        nc.sync.dma_start(out=wd_sb[:], in_=wd_d[:].bitcast(F32R))
        bias_sb = const.tile([128, 4, NJ], F32)
        nc.sync.dma_start(out=bias_sb[:], in_=bias_d[:])
        bdb_sb = const.tile([128, L], F32)
        nc.sync.dma_start(out=bdb_sb[:], in_=bdb_d[:])

        def gso_layer(g, stat_blocks, nfc, xa_out):
            # xa.T[fchunk] = sum_mt stat[mt,fchunk].T @ aT[mt]   (feature-major out)
            for fc in range(nfc):
                ps = ps_gso.tile([128, N], F32, tag="gso", name=f"psg_{g}_{fc}")
                for mt in range(NMT):
                    nc.tensor.matmul(
                        ps[:],
                        stat_blocks(mt, fc),
                        at_sb[:, mt, :],
                        start=(mt == 0),
                        stop=(mt == NMT - 1),
                    )
                nc.scalar.copy(xa_out[:, fc, :], ps[:])

        def chan_layer(gs, li, u_d, nf, moving, hT_out):
            # Z.T[jj] = sum_f U[f,jj].T @ xcatT[f] ; relu+bias ; sum K stacks.
            # f outer / graph inner so the stationary U block is reused for
            # both graphs of the pair back-to-back.
            for cc in range(NCC):
                rk = {g: [] for g in gs}
                for k in range(K):
                    jj = k * NCC + cc
                    ub = upool.tile([128, nf, 128], F32R, tag="ublk",
                                    name=f"ublk_l{li}_{jj}")
                    nc.sync.dma_start(out=ub[:], in_=u_d[jj].bitcast(F32R))
                    pss = {
                        g: ps_chan.tile([128, 512], F32, tag="chan",
                                        name=f"psc_{g}_{jj}")
                        for g in gs
                    }
                    for g in gs:
                        for f in range(nf):
                            nc.tensor.matmul(
                                pss[g][:, :N],
                                ub[:, f, :],
                                moving(g, f),
                                start=(f == 0),
                                stop=(f == nf - 1),
                            )
                    for g in gs:
                        r = rpool.tile([128, N], F32, tag=f"r{g % 2}_{k}",
                                       name=f"r_{g}_{k}")
                        nc.scalar.activation(
                            r[:], pss[g][:, :N], RELU,
                            bias=bias_sb[:, li, jj : jj + 1],
                        )
                        rk[g].append(r)
                for g in gs:
                    nc.vector.tensor_add(hT_out[g][:, cc, :N], rk[g][0][:], rk[g][1][:])
                    nc.vector.tensor_add(
                        hT_out[g][:, cc, :N], hT_out[g][:, cc, :N], rk[g][2][:]
                    )

        def transpose_h(hT, h_node):
            # hT [128, NCC, 512] (cols 400:512 zeroed) -> h_node [128, NMT, NCC, 128]
            for cc in range(NCC):
                for mt in range(NMT):
                    ps = ps_tr.tile([128, 128], F32, tag="tr")
                    nc.tensor.transpose(
                        ps[:], hT[:, cc, mt * 128 : (mt + 1) * 128].bitcast(F32),
                        ident[:],
                    )
                    nc.vector.tensor_copy(h_node[:, mt, cc, :], ps[:])

        rep_ctx = tc.For_i(0, reps, 1) if reps > 1 else nullcontext()
        with rep_ctx:
         for pp in range(G // 2):
            gs = (2 * pp, 2 * pp + 1)

            # ---------- layer 1 ----------
            xt_sb, xn_sb, xa1, hT, hn = {}, {}, {}, {}, {}
            for g in gs:
                xt_sb[g] = xpool.tile([128, 2, N], F32R, tag=f"xt{g % 2}",
                                      name=f"xt_g{g}")
                nc.sync.dma_start(out=xt_sb[g][:], in_=xt_d[g].bitcast(F32R))
                xn_sb[g] = xpool.tile([128, NMT, 2, 128], F32R, tag=f"xn{g % 2}",
                                      name=f"xn_g{g}")
                nc.sync.dma_start(out=xn_sb[g][:], in_=xn_d[g].bitcast(F32R))
                xa1[g] = xpool.tile([128, NCC, N], F32R, tag=f"xa_{g % 2}",
                                    name=f"xa1_g{g}")
                gso_layer(g, lambda mt, fc, g=g: xn_sb[g][:, mt, fc, :], 2, xa1[g])
                hT[g] = hpool.tile([128, NCC, 512], F32R, tag=f"hT{g % 2}",
                                   name=f"hT1_g{g}")
            chan_layer(
                gs, 0, u_ds[0], 4,
                lambda g, f: xt_sb[g][:, f, :] if f < 2 else xa1[g][:, f - 2, :],
                hT,
            )
            for g in gs:
                for cc in range(NCC):
                    nc.vector.memset(hT[g][:, cc, N:].bitcast(F32), 0.0)
                hn[g] = hpool.tile([128, NMT, NCC, 128], F32R, tag=f"hn{g % 2}",
                                   name=f"hn1_g{g}")
                transpose_h(hT[g], hn[g])

            # ---------- layers 2..4 ----------
            for li, u_d in zip((1, 2, 3), u_ds[1:]):
                xa, hT_new, hn_new = {}, {}, {}
                for g in gs:
                    xa[g] = xpool.tile([128, NCC, N], F32R, tag=f"xa_{g % 2}",
                                       name=f"xa_g{g}_l{li}")
                    gso_layer(g, lambda mt, cc, g=g: hn[g][:, mt, cc, :], NCC, xa[g])
                    hT_new[g] = hpool.tile([128, NCC, 512], F32R, tag=f"hT{g % 2}",
                                           name=f"hT_g{g}_l{li}")
                chan_layer(
                    gs, li, u_d, 8,
                    lambda g, f: hT[g][:, f, :N] if f < NCC else xa[g][:, f - NCC, :],
                    hT_new,
                )
                if li < 3:
                    for g in gs:
                        for cc in range(NCC):
                            nc.vector.memset(hT_new[g][:, cc, N:].bitcast(F32), 0.0)
                        hn_new[g] = hpool.tile([128, NMT, NCC, 128], F32R,
                                               tag=f"hn{g % 2}", name=f"hn_g{g}_l{li}")
                        transpose_h(hT_new[g], hn_new[g])
                    hn = hn_new
                hT = hT_new

            # ---------- dense head ----------
            for g in gs:
                for nt in range(NMT):
                    w = 128 if nt < 3 else N - 3 * 128
                    ps = ps_chan.tile([128, 512], F32, tag="chan",
                                      name=f"psy_{g}_{nt}")
                    for cc in range(NCC):
                        nc.tensor.matmul(
                            ps[:w, :L],
                            hT[g][:, cc, nt * 128 : nt * 128 + w],
                            wd_sb[:, cc, :],
                            start=(cc == 0),
                            stop=(cc == NCC - 1),
                        )
                    y_sb = ypool.tile([128, L], F32, tag="y", name=f"y_{g}_{nt}")
                    nc.vector.tensor_add(y_sb[:w, :], ps[:w, :L], bdb_sb[:w, :])
                    nc.sync.dma_start(
                        out=y_d[g, nt * 128 : nt * 128 + w, :], in_=y_sb[:w, :]
                    )

    nc.compile()
    return nc


def _pack_inputs(x, a, Ws, Vs, bs, Wd, bd):
    """Host-side packing into the per-core DMA-friendly layouts."""
    x = np.asarray(x, np.float32)
    a = np.asarray(a, np.float32)

    # aT padded: [p, mt, n] = a[n, mt*128+p], zero for node >= 400
    at_pack = np.zeros((NP, N), np.float32)
    at_pack[:N, :] = a.T
    at_pack = np.ascontiguousarray(at_pack.reshape(NMT, 128, N).transpose(1, 0, 2))

    # U1: rows [V1 pad 256 ; W1 pad 256], cols k-major (k*512 + c)
    U1 = np.zeros((512, K * C), np.float32)
    for k in range(K):
        U1[:F, k * C : (k + 1) * C] = Vs[0][k]
        U1[FP : FP + F, k * C : (k + 1) * C] = Ws[0][k]
    u_packs = [
        np.ascontiguousarray(U1.reshape(4, 128, NJ, 128).transpose(2, 1, 0, 3))
    ]
    for l in range(1, 4):
        U = np.empty((2 * C, K * C), np.float32)
        for k in range(K):
            U[:C, k * C : (k + 1) * C] = Vs[l][k] / 3.0
            U[C:, k * C : (k + 1) * C] = Ws[l][k] / 3.0
        u_packs.append(
            np.ascontiguousarray(U.reshape(8, 128, NJ, 128).transpose(2, 1, 0, 3))
        )

    wd_pack = np.ascontiguousarray(
        (np.asarray(Wd, np.float32) / 3.0).reshape(NCC, 128, L).transpose(1, 0, 2)
    )

    bias_pack = np.zeros((128, 4, NJ), np.float32)
    for li in range(4):
        for jj in range(NJ):
            k, cc = divmod(jj, NCC)
            bias_pack[:, li, jj] = bs[li][k, cc * 128 : (cc + 1) * 128]

    bdb_pack = np.ascontiguousarray(
        np.broadcast_to(np.asarray(bd, np.float32), (128, L))
    )

    in_maps = []
    for c in range(NCORES):
        xs = x[c * G : (c + 1) * G]  # (G, 400, 240)
        x_pad = np.zeros((G, NP, FP), np.float32)
        x_pad[:, :N, :F] = xs
        xn_pack = np.ascontiguousarray(
            x_pad.reshape(G, NMT, 128, 2, 128).transpose(0, 2, 1, 3, 4)
        )
        xt_pack = np.ascontiguousarray(
            x_pad[:, :N, :].transpose(0, 2, 1).reshape(G, 2, 128, N).transpose(0, 2, 1, 3)
        )
        in_maps.append(
            {
                "xt": xt_pack,
                "xn": xn_pack,
                "at": at_pack,
                "u1": u_packs[0],
                "u2": u_packs[1],
                "u3": u_packs[2],
                "u4": u_packs[3],
                "wd": wd_pack,
                "bias": bias_pack,
                "bdb": bdb_pack,
            }
        )
    return in_maps


_NC_CACHE = {}


def _get_nc(reps=1):
    key = f"nc{reps}"
    if key not in _NC_CACHE:
        _NC_CACHE[key] = _build_nc(reps)
    return _NC_CACHE[key]


def kernel(
    x, a, W1, V1, b1, W2, V2, b2, W3, V3, b3, W4, V4, b4, Wd, bd
) -> np.ndarray:
    in_maps = _pack_inputs(
        x,
        a,
        [np.asarray(W, np.float32) for W in (W1, W2, W3, W4)],
        [np.asarray(V, np.float32) for V in (V1, V2, V3, V4)],
        [np.asarray(b, np.float32) for b in (b1, b2, b3, b4)],
        Wd,
        bd,
    )
    nc = _get_nc()
    res = run_bass_kernel_spmd(nc, in_maps, core_ids=list(range(NCORES)))
    return np.concatenate([res.results[c]["y"] for c in range(NCORES)], axis=0)
